# revision 24
# baseline (speedup 1.0000x reference)
"""AttnDecoderRNN single-step decoder on 8 TRN2 NeuronCores (Bass/Tile).

Math notes (vs the reference):
  - The attention score is a single scalar broadcast over all S timesteps, so
    softmax over it is exactly uniform = 1/S (exact in f32: 1/8192 = 2^-13).
    attn_applied is therefore the mean of encoder_outputs rows, and
    attn_w/attn_b cannot affect any output.
  - The embedding lookup selects one row; done host-side as part of sharding.

Sharding (8 cores):
  - encoder_outputs sequence-sharded -> local partial row-sum -> AllReduce [H].
  - comb_w output-row-sharded (each core computes its x chunk [128]).
  - w_ih/w_hh contraction-sharded (each core uses its x/h0 chunk) ->
    AllReduce of gi/gh partials [6H]; gate math replicated on every core.
  - out_w.T vocab-sharded [H, 6400] per core (host pre-transposed + padded);
    log-softmax normalization via AllGather of per-core (max, sumexp).
"""

import numpy as np

H = 1024
V = 50257
S = 8192
NCORES = 8
PB = 128                 # partition block
HC = H // PB             # 8 h-chunks
SS = S // NCORES         # 1024 sequence rows per core
VC = 6400                # padded vocab columns per core
VPAD = VC * NCORES       # 51200
MT = VC // PB            # 50 vocab tiles of 128 per core
PAD_NEG = -30000.0       # pad bias: exp(PAD_NEG - max) == 0 in f32
WT_BUFS = 7              # out_w stream prefetch depth (half-blocks)
LCOLS = 3072             # left half-block columns (right = VC - LCOLS = 3328)

_CACHE = {}


def build_nc():
    import concourse.bacc as bacc
    import concourse.mybir as mybir
    import concourse.tile as tile
    from concourse.tile_rust import add_dep_helper

    f32 = mybir.dt.float32
    AF = mybir.ActivationFunctionType
    ALU = mybir.AluOpType
    AX = mybir.AxisListType

    nc = bacc.Bacc(
        "TRN2",
        target_bir_lowering=False,
        debug=False,
        num_devices=NCORES,
    )

    emb_d = nc.dram_tensor("emb", [H], f32, kind="ExternalInput")
    h0_d = nc.dram_tensor("h0", [H], f32, kind="ExternalInput")
    h0c_d = nc.dram_tensor("h0c", [PB], f32, kind="ExternalInput")
    enc_d = nc.dram_tensor("enc", [SS, H], f32, kind="ExternalInput")
    combt_d = nc.dram_tensor("combt", [2 * H, PB], f32, kind="ExternalInput")
    combbc_d = nc.dram_tensor("combbc", [PB], f32, kind="ExternalInput")
    wiht_d = nc.dram_tensor("wiht", [PB, 3 * H], f32, kind="ExternalInput")
    whht_d = nc.dram_tensor("whht", [PB, 3 * H], f32, kind="ExternalInput")
    bvec_d = nc.dram_tensor("bvec", [4 * H], f32, kind="ExternalInput")
    wt_d = nc.dram_tensor("wt", [H, VC], f32, kind="ExternalInput")
    obc_d = nc.dram_tensor("obc", [VC], f32, kind="ExternalInput")
    outl_d = nc.dram_tensor("out_l", [VC], f32, kind="ExternalOutput")
    outh_d = nc.dram_tensor("out_h", [H], f32, kind="ExternalOutput")

    RG = [list(range(NCORES))]

    with tile.TileContext(nc) as tc:
        with (
            tc.tile_pool(name="dram", bufs=1, space="DRAM") as dpool,
            tc.tile_pool(name="const", bufs=1) as cpool,
            tc.tile_pool(name="vecs", bufs=1) as vpool,
            tc.tile_pool(name="encp", bufs=2) as encp,
            tc.tile_pool(name="wts", bufs=1) as wpool,
            tc.tile_pool(name="wtstream", bufs=WT_BUFS) as wtp,
            tc.tile_pool(name="small", bufs=1) as spool,
            tc.tile_pool(name="psACC", bufs=1, space="PSUM") as psACC,
            tc.tile_pool(name="psT", bufs=1, space="PSUM") as psT,
        ):
            attn_bo = dpool.tile([H], f32, name="attn_bo")
            attn_rd = dpool.tile([H], f32, addr_space="Shared", name="attn_rd")
            g_bo = dpool.tile([6 * H], f32, name="g_bo")
            g_rd = dpool.tile([6 * H], f32, addr_space="Shared", name="g_rd")
            st_bo = dpool.tile([2], f32, name="st_bo")
            st_al = dpool.tile([2 * NCORES], f32, addr_space="Shared", name="st_al")

            # constants
            scale_v = cpool.tile([PB, 1], f32, name="scale_v")
            nc.vector.memset(scale_v[:], 1.0 / S)
            ones_1 = cpool.tile([1, 1], f32, name="ones_1")
            nc.vector.memset(ones_1[:], 1.0)

            # ---- Phase A: local attention partial sum (mean of enc rows)
            attn_ps = psACC.tile([PB, HC], f32, name="attn_ps", tag="acc")
            NSB = SS // PB  # 8
            enc_dmas = []
            for sb in range(NSB):
                et = encp.tile([PB, H], f32, name="et")
                enc_dmas.append(
                    nc.gpsimd.dma_start(et[:], enc_d[sb * PB:(sb + 1) * PB, :])
                )
                for hb in range(HC):
                    nc.tensor.matmul(
                        attn_ps[:, hb:hb + 1],
                        et[:, hb * PB:(hb + 1) * PB],
                        scale_v[:],
                        start=(sb == 0 and hb == 0),
                        stop=(sb == NSB - 1 and hb == HC - 1),
                    )
            attn_sb = vpool.tile([PB, HC], f32, name="attn_sb")
            nc.vector.tensor_copy(attn_sb[:], attn_ps[:])
            nc.gpsimd.dma_start(
                attn_bo[:].rearrange("(c p) -> p c", p=PB), attn_sb[:]
            )
            nc.gpsimd.collective_compute(
                "AllReduce", ALU.add, replica_groups=RG,
                ins=[attn_bo[:].opt()], outs=[attn_rd[:].opt()],
            )
            attn_v = vpool.tile([PB, HC], f32, name="attn_v")
            nc.gpsimd.dma_start(attn_v[:], attn_rd[:].rearrange("(c p) -> p c", p=PB))

            # ---- Phase B: x chunk = relu(combT_chunk . cat(emb, attn) + b)
            emb_v = vpool.tile([PB, HC], f32, name="emb_v")
            nc.sync.dma_start(emb_v[:], emb_d[:].rearrange("(c p) -> p c", p=PB))
            combt_s = wpool.tile([PB, 16, PB], f32, name="combt_s")
            combt_dma = nc.gpsimd.dma_start(
                combt_s[:], combt_d[:].rearrange("(t p) m -> p t m", p=PB)
            )
            add_dep_helper(
                combt_dma.ins, enc_dmas[-1].ins,
                reason="stream order: enc before gru weights",
            )
            combb_v = vpool.tile([PB, 1], f32, name="combb_v")
            nc.sync.dma_start(combb_v[:], combbc_d[:].rearrange("(p o) -> p o", o=1))

            x_ps = psT.tile([PB, 1], f32, name="x_ps", tag="tb")
            for k in range(16):
                rhs = emb_v[:, k:k + 1] if k < 8 else attn_v[:, k - 8:k - 7]
                nc.tensor.matmul(
                    x_ps[:], combt_s[:, k, :], rhs,
                    start=(k == 0), stop=(k == 15),
                )
            x_v = vpool.tile([PB, 1], f32, name="x_v")
            nc.scalar.activation(x_v[:], x_ps[:], AF.Relu, bias=combb_v[:])

            # ---- Phase C: GRU gate partial projections (contraction-sharded)
            # gi rows vertical: g_ps[:, r] = w_ih[r*128:(r+1)*128, chunk] @ x_chunk
            # gh rows vertical: g_ps[:, 24+r] likewise with h0 chunk.
            wih_s = wpool.tile([PB, 3 * H], f32, name="wih_s")
            wih_dma = nc.gpsimd.dma_start(wih_s[:], wiht_d[:])
            add_dep_helper(wih_dma.ins, combt_dma.ins, reason="stream order")
            whh_s = wpool.tile([PB, 3 * H], f32, name="whh_s")
            whh_dma = nc.gpsimd.dma_start(whh_s[:], whht_d[:])
            add_dep_helper(whh_dma.ins, wih_dma.ins, reason="stream order")
            h0c_v = vpool.tile([PB, 1], f32, name="h0c_v")
            nc.sync.dma_start(h0c_v[:], h0c_d[:].rearrange("(p o) -> p o", o=1))

            NRT_ = (3 * H) // PB  # 24 row-tiles per projection
            g_ps = psACC.tile([PB, 2 * NRT_], f32, name="g_ps", tag="acc")
            for r in range(NRT_):
                nc.tensor.matmul(
                    g_ps[:, r:r + 1],
                    wih_s[:, r * PB:(r + 1) * PB],
                    x_v[:],
                    start=(r == 0), stop=False,
                )
            for r in range(NRT_):
                nc.tensor.matmul(
                    g_ps[:, NRT_ + r:NRT_ + r + 1],
                    whh_s[:, r * PB:(r + 1) * PB],
                    h0c_v[:],
                    start=False, stop=(r == NRT_ - 1),
                )
            g_sb = vpool.tile([PB, 2 * NRT_], f32, name="g_sb")
            nc.vector.tensor_copy(g_sb[:], g_ps[:])
            nc.gpsimd.dma_start(
                g_bo[:].rearrange("(c p) -> p c", p=PB), g_sb[:]
            )
            nc.gpsimd.collective_compute(
                "AllReduce", ALU.add, replica_groups=RG,
                ins=[g_bo[:].opt()], outs=[g_rd[:].opt()],
            )

            # ---- Phase D: full gate math, replicated on every core
            g_all = vpool.tile([PB, 6, HC], f32, name="g_all")
            nc.gpsimd.dma_start(
                g_all[:], g_rd[:].rearrange("(s c p) -> p s c", p=PB, c=HC)
            )
            b_all = vpool.tile([PB, 4, HC], f32, name="b_all")
            nc.sync.dma_start(
                b_all[:], bvec_d[:].rearrange("(s c p) -> p s c", p=PB, c=HC)
            )
            h0_v = vpool.tile([PB, HC], f32, name="h0_v")
            nc.sync.dma_start(h0_v[:], h0_d[:].rearrange("(c p) -> p c", p=PB))

            gi_r, gi_z, gi_n = g_all[:, 0, :], g_all[:, 1, :], g_all[:, 2, :]
            gh_r, gh_z, gh_n = g_all[:, 3, :], g_all[:, 4, :], g_all[:, 5, :]
            b_r, b_z = b_all[:, 0, :], b_all[:, 1, :]
            b_in, b_hn = b_all[:, 2, :], b_all[:, 3, :]

            t_r = spool.tile([PB, HC], f32, name="t_r")
            nc.vector.tensor_add(t_r[:], gi_r, gh_r)
            nc.vector.tensor_add(t_r[:], t_r[:], b_r)
            r_g = spool.tile([PB, HC], f32, name="r_g")
            nc.scalar.activation(r_g[:], t_r[:], AF.Sigmoid)

            t_z = spool.tile([PB, HC], f32, name="t_z")
            nc.vector.tensor_add(t_z[:], gi_z, gh_z)
            nc.vector.tensor_add(t_z[:], t_z[:], b_z)
            z_g = spool.tile([PB, HC], f32, name="z_g")
            nc.scalar.activation(z_g[:], t_z[:], AF.Sigmoid)

            t_n = spool.tile([PB, HC], f32, name="t_n")
            nc.vector.tensor_add(t_n[:], gh_n, b_hn)
            nc.vector.tensor_mul(t_n[:], r_g[:], t_n[:])
            nc.vector.tensor_add(t_n[:], t_n[:], gi_n)
            nc.vector.tensor_add(t_n[:], t_n[:], b_in)
            n_g = spool.tile([PB, HC], f32, name="n_g")
            nc.scalar.activation(n_g[:], t_n[:], AF.Tanh)

            d_t = spool.tile([PB, HC], f32, name="d_t")
            nc.vector.tensor_sub(d_t[:], h0_v[:], n_g[:])
            nc.vector.tensor_mul(d_t[:], z_g[:], d_t[:])
            hnew = spool.tile([PB, HC], f32, name="hnew")
            nc.vector.tensor_add(hnew[:], n_g[:], d_t[:])
            nc.sync.dma_start(outh_d[:].rearrange("(c p) -> p c", p=PB), hnew[:])

            # ---- Phase E: vocab-sharded output projection (the big matvec)
            # h chunk is the stationary operand (1 column), weight blocks
            # stream as the moving operand at N<=512.  The horizontal
            # [1, VC] result is spread over partitions {0,32,64,96} of four
            # [128, 512] PSUM tiles via tile_position col-tiling.
            NSLC = (VC + 511) // 512  # 13 slices
            obc_r = spool.tile([1, VC], f32, name="obc_r", tag="bigrow1")
            nc.sync.dma_start(obc_r[:], obc_d[:].rearrange("(o f) -> o f", o=1))
            ps_rows = [
                psACC.tile([PB, 512], f32, name=f"ps_row{t}", tag=f"psrow{t}")
                for t in range(4)
            ]

            def slice_out(s, ncols):
                pb = 32 * (s % 4)
                return ps_rows[s // 4][pb:pb + 1, 0:ncols], (0, pb)

            # bias seeds each accumulation group (start=True)
            for s in range(NSLC):
                c0, c1 = s * 512, min((s + 1) * 512, VC)
                out_ap, tpos = slice_out(s, c1 - c0)
                nc.tensor.matmul(
                    out_ap, ones_1[:], obc_r[:, c0:c1],
                    start=True, stop=False, tile_position=tpos,
                )
            # weight stream: 16 half-blocks (cols [0,LCOLS) and [LCOLS,VC))
            n_wt = 0
            for kb in range(HC):
                for (base, width, s_lo, s_hi) in (
                    (0, LCOLS, 0, LCOLS // 512),
                    (LCOLS, VC - LCOLS, LCOLS // 512, NSLC),
                ):
                    wt_t = wtp.tile([PB, width], f32, name="wt_t", tag="wt_t")
                    wdma = nc.gpsimd.dma_start(
                        wt_t[:],
                        wt_d[kb * PB:(kb + 1) * PB, base:base + width],
                    )
                    if n_wt < WT_BUFS:
                        add_dep_helper(
                            wdma.ins, whh_dma.ins,
                            reason="stream order: small weights before out_w",
                        )
                    n_wt += 1
                    for s in range(s_lo, s_hi):
                        c0, c1 = s * 512, min((s + 1) * 512, VC)
                        out_ap, tpos = slice_out(s, c1 - c0)
                        nc.tensor.matmul(
                            out_ap, hnew[:, kb:kb + 1],
                            wt_t[:, c0 - base:c1 - base],
                            start=False, stop=(kb == HC - 1),
                            tile_position=tpos,
                        )

            # assemble logits row + per-slice maxes (ACT copies / DVE maxes)
            lrow = spool.tile([1, VC], f32, name="lrow", tag="bigrow1")
            smax = spool.tile([1, NSLC], f32, name="smax")
            for s in range(NSLC):
                c0, c1 = s * 512, min((s + 1) * 512, VC)
                out_ap, _ = slice_out(s, c1 - c0)
                nc.scalar.copy(lrow[:, c0:c1], out_ap)
                nc.vector.tensor_reduce(
                    smax[:, s:s + 1], out_ap, axis=AX.X, op=ALU.max
                )
            mval = spool.tile([1, 1], f32, name="mval")
            nc.vector.tensor_reduce(mval[:], smax[:], axis=AX.X, op=ALU.max)
            negm = spool.tile([1, 1], f32, name="negm")
            nc.vector.tensor_scalar_mul(negm[:], mval[:], -1.0)

            # local sum(exp(logit - max))
            etile = spool.tile([1, VC], f32, name="etile", tag="bigrow2")
            sval = spool.tile([1, 1], f32, name="sval")
            nc.scalar.activation(
                etile[:], lrow[:], AF.Exp, bias=negm[:], accum_out=sval[:]
            )

            nc.gpsimd.dma_start(st_bo[0:1].rearrange("(o f) -> o f", o=1), mval[:])
            nc.gpsimd.dma_start(st_bo[1:2].rearrange("(o f) -> o f", o=1), sval[:])
            nc.gpsimd.collective_compute(
                "AllGather", ALU.bypass, replica_groups=RG,
                ins=[st_bo[:].opt()], outs=[st_al[:].opt()],
            )

            # global logsumexp from the 8 (max, sumexp) pairs
            m_row = spool.tile([1, NCORES], f32, name="m_row")
            nc.gpsimd.dma_start(
                m_row[:], st_al[:].rearrange("(r t) -> t r", t=2)[0:1, :]
            )
            s_row = spool.tile([1, NCORES], f32, name="s_row")
            nc.gpsimd.dma_start(
                s_row[:], st_al[:].rearrange("(r t) -> t r", t=2)[1:2, :]
            )
            Mv = spool.tile([1, 1], f32, name="Mv")
            nc.vector.tensor_reduce(Mv[:], m_row[:], axis=AX.X, op=ALU.max)
            negM = spool.tile([1, 1], f32, name="negM")
            nc.vector.tensor_scalar_mul(negM[:], Mv[:], -1.0)
            wrow = spool.tile([1, NCORES], f32, name="wrow")
            nc.scalar.activation(wrow[:], m_row[:], AF.Exp, bias=negM[:])
            nc.vector.tensor_mul(wrow[:], wrow[:], s_row[:])
            Sv = spool.tile([1, 1], f32, name="Sv")
            nc.vector.tensor_reduce(Sv[:], wrow[:], axis=AX.X, op=ALU.add)
            lnS = spool.tile([1, 1], f32, name="lnS")
            nc.scalar.activation(lnS[:], Sv[:], AF.Ln)
            lse = spool.tile([1, 1], f32, name="lse")
            nc.vector.tensor_add(lse[:], lnS[:], Mv[:])
            nc.vector.tensor_scalar_mul(lse[:], lse[:], -1.0)

            final = spool.tile([1, VC], f32, name="final", tag="bigrow2")
            nc.scalar.activation(final[:], lrow[:], AF.Identity, bias=lse[:])
            nc.sync.dma_start(outl_d[:].rearrange("(o f) -> o f", o=1), final[:])

    nc.compile()
    return nc


def _get_nc():
    if "nc" not in _CACHE:
        _CACHE["nc"] = build_nc()
    return _CACHE["nc"]


def prep_in_maps(inputs):
    f = np.float32
    idx = int(np.asarray(inputs["input"]).reshape(-1)[0])
    emb = np.ascontiguousarray(np.asarray(inputs["embedding"])[idx], dtype=f)
    h0 = np.ascontiguousarray(np.asarray(inputs["hidden"]).reshape(H), dtype=f)
    enc = np.ascontiguousarray(np.asarray(inputs["encoder_outputs"]), dtype=f)
    combt = np.ascontiguousarray(np.asarray(inputs["comb_w"]).T, dtype=f)
    wiht = np.ascontiguousarray(np.asarray(inputs["w_ih"]).T, dtype=f)
    whht = np.ascontiguousarray(np.asarray(inputs["w_hh"]).T, dtype=f)
    b_ih = np.asarray(inputs["b_ih"], dtype=f)
    b_hh = np.asarray(inputs["b_hh"], dtype=f)
    bvec = np.concatenate([
        b_ih[:H] + b_hh[:H],
        b_ih[H:2 * H] + b_hh[H:2 * H],
        b_ih[2 * H:],
        b_hh[2 * H:],
    ]).astype(f)
    comb_b = np.asarray(inputs["comb_b"], dtype=f)
    wt_pad = np.zeros((H, VPAD), dtype=f)
    wt_pad[:, :V] = np.asarray(inputs["out_w"], dtype=f).T
    ob_pad = np.full((VPAD,), PAD_NEG, dtype=f)
    ob_pad[:V] = np.asarray(inputs["out_b"], dtype=f)

    in_maps = []
    for i in range(NCORES):
        in_maps.append({
            "emb": emb,
            "h0": h0,
            "h0c": np.ascontiguousarray(h0[i * PB:(i + 1) * PB]),
            "enc": np.ascontiguousarray(enc[i * SS:(i + 1) * SS]),
            "combt": np.ascontiguousarray(combt[:, i * PB:(i + 1) * PB]),
            "combbc": np.ascontiguousarray(comb_b[i * PB:(i + 1) * PB]),
            "wiht": np.ascontiguousarray(wiht[i * PB:(i + 1) * PB]),
            "whht": np.ascontiguousarray(whht[i * PB:(i + 1) * PB]),
            "bvec": bvec,
            "wt": np.ascontiguousarray(wt_pad[:, i * VC:(i + 1) * VC]),
            "obc": np.ascontiguousarray(ob_pad[i * VC:(i + 1) * VC]),
        })
    return in_maps


def assemble_outputs(results):
    logits = np.concatenate(
        [np.asarray(results[i]["out_l"]).reshape(-1) for i in range(NCORES)]
    )[:V]
    out = np.ascontiguousarray(logits, dtype=np.float32)[None, :]
    h_new = np.ascontiguousarray(
        np.asarray(results[0]["out_h"]).reshape(-1), dtype=np.float32
    )[None, None, :]
    attn_w = np.full((1, S), np.float32(1.0 / S), dtype=np.float32)
    return out, h_new, attn_w


def run_traced(inputs):
    """Run on HW with NTFF profiling; returns (outputs_tuple, exec_time_ns)."""
    from concourse.bass_utils import run_bass_kernel_spmd
    nc = _get_nc()
    in_maps = prep_in_maps(inputs)
    br = run_bass_kernel_spmd(
        nc, in_maps, core_ids=list(range(NCORES)), trace=True
    )
    return assemble_outputs(br.results), br.exec_time_ns


def kernel(**inputs):
    from concourse.bass_utils import run_bass_kernel_spmd
    nc = _get_nc()
    in_maps = prep_in_maps(inputs)
    br = run_bass_kernel_spmd(nc, in_maps, core_ids=list(range(NCORES)))
    return assemble_outputs(br.results)


# revision 26
# speedup vs baseline: 1.3062x; 1.3062x over previous
"""AttnDecoderRNN single-step decoder on 8 TRN2 NeuronCores (Bass/Tile).

Math notes (vs the reference):
  - The attention score is a single scalar broadcast over all S timesteps, so
    softmax over it is exactly uniform = 1/S (exact in f32: 1/8192 = 2^-13).
    attn_applied is therefore the mean of encoder_outputs rows, and
    attn_w/attn_b cannot affect any output.
  - The embedding lookup selects one row; done host-side as part of sharding.

Sharding (8 cores):
  - encoder_outputs sequence-sharded -> local partial row-sum -> AllReduce [H].
  - comb_w output-row-sharded (each core computes its x chunk [128]).
  - w_ih/w_hh contraction-sharded (each core uses its x/h0 chunk) ->
    AllReduce of gi/gh partials [6H]; gate math replicated on every core.
  - out_w.T vocab-sharded [H, 6400] per core (host pre-transposed + padded);
    log-softmax normalization via AllGather of per-core (max, sumexp).
"""

import numpy as np

H = 1024
V = 50257
S = 8192
NCORES = 8
PB = 128                 # partition block
HC = H // PB             # 8 h-chunks
SS = S // NCORES         # 1024 sequence rows per core
VC = 6400                # padded vocab columns per core
VPAD = VC * NCORES       # 51200
MT = VC // PB            # 50 vocab tiles of 128 per core
PAD_NEG = -30000.0       # pad bias: exp(PAD_NEG - max) == 0 in f32
WT_BUFS = 7              # out_w stream prefetch depth (half-blocks)
LCOLS = 3072             # left half-block columns (right = VC - LCOLS = 3328)

_CACHE = {}


def build_nc():
    import concourse.bacc as bacc
    import concourse.mybir as mybir
    import concourse.tile as tile
    from concourse.tile_rust import add_dep_helper

    f32 = mybir.dt.float32
    AF = mybir.ActivationFunctionType
    ALU = mybir.AluOpType
    AX = mybir.AxisListType

    nc = bacc.Bacc(
        "TRN2",
        target_bir_lowering=False,
        debug=False,
        num_devices=NCORES,
    )

    emb_d = nc.dram_tensor("emb", [H], f32, kind="ExternalInput")
    h0_d = nc.dram_tensor("h0", [H], f32, kind="ExternalInput")
    h0c_d = nc.dram_tensor("h0c", [PB], f32, kind="ExternalInput")
    enc_d = nc.dram_tensor("enc", [SS, H], f32, kind="ExternalInput")
    combt_d = nc.dram_tensor("combt", [PB, 16 * PB], f32, kind="ExternalInput")
    combbc_d = nc.dram_tensor("combbc", [PB], f32, kind="ExternalInput")
    wiht_d = nc.dram_tensor("wiht", [PB, 3 * H], f32, kind="ExternalInput")
    whht_d = nc.dram_tensor("whht", [PB, 3 * H], f32, kind="ExternalInput")
    bvec_d = nc.dram_tensor("bvec", [4 * H], f32, kind="ExternalInput")
    wt_d = nc.dram_tensor("wt", [H, VC], f32, kind="ExternalInput")
    obc_d = nc.dram_tensor("obc", [VC], f32, kind="ExternalInput")
    outl_d = nc.dram_tensor("out_l", [VC], f32, kind="ExternalOutput")
    outh_d = nc.dram_tensor("out_h", [H], f32, kind="ExternalOutput")

    RG = [list(range(NCORES))]

    with tile.TileContext(nc) as tc:
        with (
            tc.tile_pool(name="dram", bufs=1, space="DRAM") as dpool,
            tc.tile_pool(name="const", bufs=1) as cpool,
            tc.tile_pool(name="vecs", bufs=1) as vpool,
            tc.tile_pool(name="encp", bufs=2) as encp,
            tc.tile_pool(name="wts", bufs=1) as wpool,
            tc.tile_pool(name="wtstream", bufs=WT_BUFS) as wtp,
            tc.tile_pool(name="small", bufs=1) as spool,
            tc.tile_pool(name="psACC", bufs=1, space="PSUM") as psACC,
            tc.tile_pool(name="psT", bufs=1, space="PSUM") as psT,
        ):
            attn_bo = dpool.tile([H], f32, name="attn_bo")
            attn_rd = dpool.tile([H], f32, addr_space="Shared", name="attn_rd")
            g_bo = dpool.tile([6 * H], f32, name="g_bo")
            g_rd = dpool.tile([6 * H], f32, addr_space="Shared", name="g_rd")
            st_bo = dpool.tile([2], f32, name="st_bo")
            st_al = dpool.tile([2 * NCORES], f32, addr_space="Shared", name="st_al")

            # constants
            scale_v = cpool.tile([PB, 1], f32, name="scale_v")
            nc.vector.memset(scale_v[:], 1.0 / S)
            ones_1 = cpool.tile([1, 1], f32, name="ones_1")
            nc.vector.memset(ones_1[:], 1.0)

            # ---- Phase A: local attention partial sum (mean of enc rows)
            attn_ps = psACC.tile([PB, HC], f32, name="attn_ps", tag="acc")
            NSB = SS // PB  # 8
            enc_dmas = []
            for sb in range(NSB):
                et = encp.tile([PB, H], f32, name="et")
                enc_dmas.append(
                    nc.gpsimd.dma_start(et[:], enc_d[sb * PB:(sb + 1) * PB, :])
                )
                for hb in range(HC):
                    nc.tensor.matmul(
                        attn_ps[:, hb:hb + 1],
                        et[:, hb * PB:(hb + 1) * PB],
                        scale_v[:],
                        start=(sb == 0 and hb == 0),
                        stop=(sb == NSB - 1 and hb == HC - 1),
                    )
            attn_sb = vpool.tile([PB, HC], f32, name="attn_sb")
            nc.vector.tensor_copy(attn_sb[:], attn_ps[:])
            nc.gpsimd.dma_start(
                attn_bo[:].rearrange("(p c) -> p c", p=PB), attn_sb[:]
            )
            nc.gpsimd.collective_compute(
                "AllReduce", ALU.add, replica_groups=RG,
                ins=[attn_bo[:].opt()], outs=[attn_rd[:].opt()],
            )
            attn_v = vpool.tile([PB, HC], f32, name="attn_v")
            nc.gpsimd.dma_start(attn_v[:], attn_rd[:].rearrange("(p c) -> p c", p=PB))

            # ---- Phase B: x chunk = relu(combT_chunk . cat(emb, attn) + b)
            emb_v = vpool.tile([PB, HC], f32, name="emb_v")
            nc.sync.dma_start(emb_v[:], emb_d[:].rearrange("(p c) -> p c", p=PB))
            combt_s = wpool.tile([PB, 16 * PB], f32, name="combt_s")
            combt_dma = nc.gpsimd.dma_start(combt_s[:], combt_d[:])
            add_dep_helper(
                combt_dma.ins, enc_dmas[-1].ins,
                reason="stream order: enc before gru weights",
            )
            combb_v = vpool.tile([PB, 1], f32, name="combb_v")
            nc.sync.dma_start(combb_v[:], combbc_d[:].rearrange("(p o) -> p o", o=1))

            x_ps = psT.tile([PB, 1], f32, name="x_ps", tag="tb")
            for k in range(16):
                rhs = emb_v[:, k:k + 1] if k < 8 else attn_v[:, k - 8:k - 7]
                nc.tensor.matmul(
                    x_ps[:], combt_s[:, k * PB:(k + 1) * PB], rhs,
                    start=(k == 0), stop=(k == 15),
                )
            x_v = vpool.tile([PB, 1], f32, name="x_v")
            nc.scalar.activation(x_v[:], x_ps[:], AF.Relu, bias=combb_v[:])

            # ---- Phase C: GRU gate partial projections (contraction-sharded)
            # gi rows vertical: g_ps[:, r] = w_ih[r*128:(r+1)*128, chunk] @ x_chunk
            # gh rows vertical: g_ps[:, 24+r] likewise with h0 chunk.
            wih_s = wpool.tile([PB, 3 * H], f32, name="wih_s")
            wih_dma = nc.gpsimd.dma_start(wih_s[:], wiht_d[:])
            add_dep_helper(wih_dma.ins, combt_dma.ins, reason="stream order")
            whh_s = wpool.tile([PB, 3 * H], f32, name="whh_s")
            whh_dma = nc.gpsimd.dma_start(whh_s[:], whht_d[:])
            add_dep_helper(whh_dma.ins, wih_dma.ins, reason="stream order")
            h0c_v = vpool.tile([PB, 1], f32, name="h0c_v")
            nc.sync.dma_start(h0c_v[:], h0c_d[:].rearrange("(p o) -> p o", o=1))

            NRT_ = (3 * H) // PB  # 24 row-tiles per projection
            g_ps = psACC.tile([PB, 2 * NRT_], f32, name="g_ps", tag="acc")
            for r in range(NRT_):
                nc.tensor.matmul(
                    g_ps[:, r:r + 1],
                    wih_s[:, r * PB:(r + 1) * PB],
                    x_v[:],
                    start=(r == 0), stop=False,
                )
            for r in range(NRT_):
                nc.tensor.matmul(
                    g_ps[:, NRT_ + r:NRT_ + r + 1],
                    whh_s[:, r * PB:(r + 1) * PB],
                    h0c_v[:],
                    start=False, stop=(r == NRT_ - 1),
                )
            g_sb = vpool.tile([PB, 2 * NRT_], f32, name="g_sb")
            nc.vector.tensor_copy(g_sb[:], g_ps[:])
            nc.gpsimd.dma_start(
                g_bo[:].rearrange("(p c) -> p c", p=PB), g_sb[:]
            )
            nc.gpsimd.collective_compute(
                "AllReduce", ALU.add, replica_groups=RG,
                ins=[g_bo[:].opt()], outs=[g_rd[:].opt()],
            )

            # ---- Phase D: full gate math, replicated on every core
            g_all = vpool.tile([PB, 6, HC], f32, name="g_all")
            nc.gpsimd.dma_start(
                g_all[:], g_rd[:].rearrange("(p s c) -> p s c", p=PB, s=6)
            )
            b_all = vpool.tile([PB, 4, HC], f32, name="b_all")
            nc.sync.dma_start(
                b_all[:], bvec_d[:].rearrange("(p s c) -> p s c", p=PB, s=4)
            )
            h0_v = vpool.tile([PB, HC], f32, name="h0_v")
            nc.sync.dma_start(h0_v[:], h0_d[:].rearrange("(p c) -> p c", p=PB))

            gi_r, gi_z, gi_n = g_all[:, 0, :], g_all[:, 1, :], g_all[:, 2, :]
            gh_r, gh_z, gh_n = g_all[:, 3, :], g_all[:, 4, :], g_all[:, 5, :]
            b_r, b_z = b_all[:, 0, :], b_all[:, 1, :]
            b_in, b_hn = b_all[:, 2, :], b_all[:, 3, :]

            t_r = spool.tile([PB, HC], f32, name="t_r")
            nc.vector.tensor_add(t_r[:], gi_r, gh_r)
            nc.vector.tensor_add(t_r[:], t_r[:], b_r)
            r_g = spool.tile([PB, HC], f32, name="r_g")
            nc.scalar.activation(r_g[:], t_r[:], AF.Sigmoid)

            t_z = spool.tile([PB, HC], f32, name="t_z")
            nc.vector.tensor_add(t_z[:], gi_z, gh_z)
            nc.vector.tensor_add(t_z[:], t_z[:], b_z)
            z_g = spool.tile([PB, HC], f32, name="z_g")
            nc.scalar.activation(z_g[:], t_z[:], AF.Sigmoid)

            t_n = spool.tile([PB, HC], f32, name="t_n")
            nc.vector.tensor_add(t_n[:], gh_n, b_hn)
            nc.vector.tensor_mul(t_n[:], r_g[:], t_n[:])
            nc.vector.tensor_add(t_n[:], t_n[:], gi_n)
            nc.vector.tensor_add(t_n[:], t_n[:], b_in)
            n_g = spool.tile([PB, HC], f32, name="n_g")
            nc.scalar.activation(n_g[:], t_n[:], AF.Tanh)

            d_t = spool.tile([PB, HC], f32, name="d_t")
            nc.vector.tensor_sub(d_t[:], h0_v[:], n_g[:])
            nc.vector.tensor_mul(d_t[:], z_g[:], d_t[:])
            hnew = spool.tile([PB, HC], f32, name="hnew")
            nc.vector.tensor_add(hnew[:], n_g[:], d_t[:])
            nc.sync.dma_start(outh_d[:].rearrange("(p c) -> p c", p=PB), hnew[:])

            # ---- Phase E: vocab-sharded output projection (the big matvec)
            # h chunk is the stationary operand (1 column), weight blocks
            # stream as the moving operand at N<=512.  The horizontal
            # [1, VC] result is spread over partitions {0,32,64,96} of four
            # [128, 512] PSUM tiles via tile_position col-tiling.
            NSLC = (VC + 511) // 512  # 13 slices
            obc_r = spool.tile([1, VC], f32, name="obc_r", tag="bigrow1")
            nc.sync.dma_start(obc_r[:], obc_d[:].rearrange("(o f) -> o f", o=1))
            ps_rows = [
                psACC.tile([PB, 512], f32, name=f"ps_row{t}", tag=f"psrow{t}")
                for t in range(4)
            ]

            def slice_out(s, ncols):
                pb = 32 * (s % 4)
                return ps_rows[s // 4][pb:pb + 1, 0:ncols], (0, pb)

            # bias seeds each accumulation group (start=True)
            for s in range(NSLC):
                c0, c1 = s * 512, min((s + 1) * 512, VC)
                out_ap, tpos = slice_out(s, c1 - c0)
                nc.tensor.matmul(
                    out_ap, ones_1[:], obc_r[:, c0:c1],
                    start=True, stop=False, tile_position=tpos,
                )
            # weight stream: 16 half-blocks (cols [0,LCOLS) and [LCOLS,VC))
            n_wt = 0
            for kb in range(HC):
                for (base, width, s_lo, s_hi) in (
                    (0, LCOLS, 0, LCOLS // 512),
                    (LCOLS, VC - LCOLS, LCOLS // 512, NSLC),
                ):
                    wt_t = wtp.tile([PB, width], f32, name="wt_t", tag="wt_t")
                    wdma = nc.gpsimd.dma_start(
                        wt_t[:],
                        wt_d[kb * PB:(kb + 1) * PB, base:base + width],
                    )
                    if n_wt < WT_BUFS:
                        add_dep_helper(
                            wdma.ins, whh_dma.ins,
                            reason="stream order: small weights before out_w",
                        )
                    n_wt += 1
                    for s in range(s_lo, s_hi):
                        c0, c1 = s * 512, min((s + 1) * 512, VC)
                        out_ap, tpos = slice_out(s, c1 - c0)
                        nc.tensor.matmul(
                            out_ap, hnew[:, kb:kb + 1],
                            wt_t[:, c0 - base:c1 - base],
                            start=False, stop=(kb == HC - 1),
                            tile_position=tpos,
                        )

            # assemble logits row + per-slice maxes (ACT copies / DVE maxes)
            lrow = spool.tile([1, VC], f32, name="lrow", tag="bigrow1")
            smax = spool.tile([1, NSLC], f32, name="smax")
            for s in range(NSLC):
                c0, c1 = s * 512, min((s + 1) * 512, VC)
                out_ap, _ = slice_out(s, c1 - c0)
                nc.scalar.copy(lrow[:, c0:c1], out_ap)
                nc.vector.tensor_reduce(
                    smax[:, s:s + 1], out_ap, axis=AX.X, op=ALU.max
                )
            mval = spool.tile([1, 1], f32, name="mval")
            nc.vector.tensor_reduce(mval[:], smax[:], axis=AX.X, op=ALU.max)
            negm = spool.tile([1, 1], f32, name="negm")
            nc.vector.tensor_scalar_mul(negm[:], mval[:], -1.0)

            # local sum(exp(logit - max))
            etile = spool.tile([1, VC], f32, name="etile", tag="bigrow2")
            sval = spool.tile([1, 1], f32, name="sval")
            nc.scalar.activation(
                etile[:], lrow[:], AF.Exp, bias=negm[:], accum_out=sval[:]
            )

            nc.gpsimd.dma_start(st_bo[0:1].rearrange("(o f) -> o f", o=1), mval[:])
            nc.gpsimd.dma_start(st_bo[1:2].rearrange("(o f) -> o f", o=1), sval[:])
            nc.gpsimd.collective_compute(
                "AllGather", ALU.bypass, replica_groups=RG,
                ins=[st_bo[:].opt()], outs=[st_al[:].opt()],
            )

            # global logsumexp from the 8 (max, sumexp) pairs
            m_row = spool.tile([1, NCORES], f32, name="m_row")
            nc.gpsimd.dma_start(
                m_row[:], st_al[:].rearrange("(r t) -> t r", t=2)[0:1, :]
            )
            s_row = spool.tile([1, NCORES], f32, name="s_row")
            nc.gpsimd.dma_start(
                s_row[:], st_al[:].rearrange("(r t) -> t r", t=2)[1:2, :]
            )
            Mv = spool.tile([1, 1], f32, name="Mv")
            nc.vector.tensor_reduce(Mv[:], m_row[:], axis=AX.X, op=ALU.max)
            negM = spool.tile([1, 1], f32, name="negM")
            nc.vector.tensor_scalar_mul(negM[:], Mv[:], -1.0)
            wrow = spool.tile([1, NCORES], f32, name="wrow")
            nc.scalar.activation(wrow[:], m_row[:], AF.Exp, bias=negM[:])
            nc.vector.tensor_mul(wrow[:], wrow[:], s_row[:])
            Sv = spool.tile([1, 1], f32, name="Sv")
            nc.vector.tensor_reduce(Sv[:], wrow[:], axis=AX.X, op=ALU.add)
            lnS = spool.tile([1, 1], f32, name="lnS")
            nc.scalar.activation(lnS[:], Sv[:], AF.Ln)
            lse = spool.tile([1, 1], f32, name="lse")
            nc.vector.tensor_add(lse[:], lnS[:], Mv[:])
            nc.vector.tensor_scalar_mul(lse[:], lse[:], -1.0)

            final = spool.tile([1, VC], f32, name="final", tag="bigrow2")
            nc.scalar.activation(final[:], lrow[:], AF.Identity, bias=lse[:])
            nc.sync.dma_start(outl_d[:].rearrange("(o f) -> o f", o=1), final[:])

    nc.compile()
    return nc


def _get_nc():
    if "nc" not in _CACHE:
        _CACHE["nc"] = build_nc()
    return _CACHE["nc"]


def _pmaj(v):
    """[H]-vector -> p-major layout: out[p*C + c] = v[c*PB + p]."""
    c = v.size // PB
    return np.ascontiguousarray(v.reshape(c, PB).T).reshape(-1)


def prep_in_maps(inputs):
    f = np.float32
    idx = int(np.asarray(inputs["input"]).reshape(-1)[0])
    emb = np.ascontiguousarray(np.asarray(inputs["embedding"])[idx], dtype=f)
    h0 = np.ascontiguousarray(np.asarray(inputs["hidden"]).reshape(H), dtype=f)
    enc = np.ascontiguousarray(np.asarray(inputs["encoder_outputs"]), dtype=f)
    combt = np.ascontiguousarray(np.asarray(inputs["comb_w"]).T, dtype=f)
    wiht = np.ascontiguousarray(np.asarray(inputs["w_ih"]).T, dtype=f)
    whht = np.ascontiguousarray(np.asarray(inputs["w_hh"]).T, dtype=f)
    b_ih = np.asarray(inputs["b_ih"], dtype=f)
    b_hh = np.asarray(inputs["b_hh"], dtype=f)
    # p-major-interleaved bias sections: bvec[p, s, c] = sec_s[c*PB + p]
    secs = [b_ih[:H] + b_hh[:H], b_ih[H:2 * H] + b_hh[H:2 * H],
            b_ih[2 * H:], b_hh[2 * H:]]
    bvec = np.ascontiguousarray(
        np.stack([sec.reshape(HC, PB) for sec in secs]).transpose(2, 0, 1)
    ).reshape(-1).astype(f)
    comb_b = np.asarray(inputs["comb_b"], dtype=f)
    wt_pad = np.zeros((H, VPAD), dtype=f)
    wt_pad[:, :V] = np.asarray(inputs["out_w"], dtype=f).T
    ob_pad = np.full((VPAD,), PAD_NEG, dtype=f)
    ob_pad[:V] = np.asarray(inputs["out_b"], dtype=f)

    in_maps = []
    for i in range(NCORES):
        # combt_pm[p, k*PB + m] = combT[k*PB + p, i*PB + m]
        ct = combt[:, i * PB:(i + 1) * PB].reshape(16, PB, PB)
        ct = np.ascontiguousarray(ct.transpose(1, 0, 2)).reshape(PB, 16 * PB)
        in_maps.append({
            "emb": _pmaj(emb),
            "h0": _pmaj(h0),
            "h0c": np.ascontiguousarray(h0[i * PB:(i + 1) * PB]),
            "enc": np.ascontiguousarray(enc[i * SS:(i + 1) * SS]),
            "combt": ct,
            "combbc": np.ascontiguousarray(comb_b[i * PB:(i + 1) * PB]),
            "wiht": np.ascontiguousarray(wiht[i * PB:(i + 1) * PB]),
            "whht": np.ascontiguousarray(whht[i * PB:(i + 1) * PB]),
            "bvec": bvec,
            "wt": np.ascontiguousarray(wt_pad[:, i * VC:(i + 1) * VC]),
            "obc": np.ascontiguousarray(ob_pad[i * VC:(i + 1) * VC]),
        })
    return in_maps


def assemble_outputs(results):
    logits = np.concatenate(
        [np.asarray(results[i]["out_l"]).reshape(-1) for i in range(NCORES)]
    )[:V]
    out = np.ascontiguousarray(logits, dtype=np.float32)[None, :]
    h_pm = np.asarray(results[0]["out_h"]).reshape(PB, HC)
    h_new = np.ascontiguousarray(h_pm.T, dtype=np.float32).reshape(-1)[None, None, :]
    attn_w = np.full((1, S), np.float32(1.0 / S), dtype=np.float32)
    return out, h_new, attn_w


def run_traced(inputs):
    """Run on HW with NTFF profiling; returns (outputs_tuple, exec_time_ns)."""
    from concourse.bass_utils import run_bass_kernel_spmd
    nc = _get_nc()
    in_maps = prep_in_maps(inputs)
    br = run_bass_kernel_spmd(
        nc, in_maps, core_ids=list(range(NCORES)), trace=True
    )
    return assemble_outputs(br.results), br.exec_time_ns


def kernel(**inputs):
    from concourse.bass_utils import run_bass_kernel_spmd
    nc = _get_nc()
    in_maps = prep_in_maps(inputs)
    br = run_bass_kernel_spmd(nc, in_maps, core_ids=list(range(NCORES)))
    return assemble_outputs(br.results)


# revision 34
# speedup vs baseline: 1.3805x; 1.0569x over previous
"""AttnDecoderRNN single-step decoder on 8 TRN2 NeuronCores (Bass/Tile).

Math notes (vs the reference):
  - The attention score is a single scalar broadcast over all S timesteps, so
    softmax over it is exactly uniform = 1/S (exact in f32: 1/8192 = 2^-13).
    attn_applied is therefore the mean of encoder_outputs rows, and
    attn_w/attn_b cannot affect any output.
  - The embedding lookup selects one row; done host-side as part of sharding.

Sharding (8 cores):
  - encoder_outputs sequence-sharded -> local partial row-sum -> AllReduce [H].
  - comb_w output-row-sharded (each core computes its x chunk [128]).
  - w_ih/w_hh contraction-sharded (each core uses its x/h0 chunk) ->
    AllReduce of gi/gh partials [6H]; gate math replicated on every core.
  - out_w.T vocab-sharded [H, 6400] per core (host pre-transposed + padded);
    log-softmax normalization via AllGather of per-core (max, sumexp).
"""

import numpy as np

H = 1024
V = 50257
S = 8192
NCORES = 8
PB = 128                 # partition block
HC = H // PB             # 8 h-chunks
SS = S // NCORES         # 1024 sequence rows per core
VC = 6400                # padded vocab columns per core
VPAD = VC * NCORES       # 51200
MT = VC // PB            # 50 vocab tiles of 128 per core
PAD_NEG = -30000.0       # pad bias: exp(PAD_NEG - max) == 0 in f32
WT_BUFS = 9              # out_w stream prefetch depth (half-blocks)
LCOLS = 3072             # left half-block columns (right = VC - LCOLS = 3328)

_CACHE = {}


def build_nc():
    import concourse.bacc as bacc
    import concourse.mybir as mybir
    import concourse.tile as tile
    from concourse.tile_rust import add_dep_helper

    f32 = mybir.dt.float32
    AF = mybir.ActivationFunctionType
    ALU = mybir.AluOpType
    AX = mybir.AxisListType

    nc = bacc.Bacc(
        "TRN2",
        target_bir_lowering=False,
        debug=False,
        num_devices=NCORES,
    )

    emb_d = nc.dram_tensor("emb", [H], f32, kind="ExternalInput")
    h0_d = nc.dram_tensor("h0", [H], f32, kind="ExternalInput")
    h0c_d = nc.dram_tensor("h0c", [PB], f32, kind="ExternalInput")
    enc_d = nc.dram_tensor("enc", [SS, H], f32, kind="ExternalInput")
    combt_d = nc.dram_tensor("combt", [PB, 16 * PB], f32, kind="ExternalInput")
    combbc_d = nc.dram_tensor("combbc", [PB], f32, kind="ExternalInput")
    wiht_d = nc.dram_tensor("wiht", [PB, 3 * H], f32, kind="ExternalInput")
    whht_d = nc.dram_tensor("whht", [PB, 3 * H], f32, kind="ExternalInput")
    bvec_d = nc.dram_tensor("bvec", [4 * H], f32, kind="ExternalInput")
    wt_d = nc.dram_tensor("wt", [H, VC], f32, kind="ExternalInput")
    obc_d = nc.dram_tensor("obc", [VC], f32, kind="ExternalInput")
    outl_d = nc.dram_tensor("out_l", [VC], f32, kind="ExternalOutput")
    outh_d = nc.dram_tensor("out_h", [H], f32, kind="ExternalOutput")

    RG = [list(range(NCORES))]

    with tile.TileContext(nc) as tc:
        with (
            tc.tile_pool(name="dram", bufs=1, space="DRAM") as dpool,
            tc.tile_pool(name="const", bufs=1) as cpool,
            tc.tile_pool(name="vecs", bufs=1) as vpool,
            tc.tile_pool(name="encp", bufs=4) as encp,
            tc.tile_pool(name="wts", bufs=1) as wpool,
            tc.tile_pool(name="wtstream", bufs=WT_BUFS) as wtp,
            tc.tile_pool(name="small", bufs=1) as spool,
            tc.tile_pool(name="psACC", bufs=1, space="PSUM") as psACC,
            tc.tile_pool(name="psT", bufs=1, space="PSUM") as psT,
        ):
            attn_bo = dpool.tile([H], f32, name="attn_bo")
            attn_rd = dpool.tile([H], f32, addr_space="Shared", name="attn_rd")
            g_bo = dpool.tile([6 * H], f32, name="g_bo")
            g_rd = dpool.tile([6 * H], f32, addr_space="Shared", name="g_rd")
            st_bo = dpool.tile([2], f32, name="st_bo")
            st_al = dpool.tile([2 * NCORES], f32, addr_space="Shared", name="st_al")

            # constants
            scale_v = cpool.tile([PB, 1], f32, name="scale_v")
            nc.vector.memset(scale_v[:], 1.0 / S)
            ones_1 = cpool.tile([1, 1], f32, name="ones_1")
            nc.vector.memset(ones_1[:], 1.0)

            # ---- Phase A: local attention partial sum (mean of enc rows)
            # DVE accumulates the 8 seq blocks; PE does one ones-matvec per
            # h-chunk to fold the 128 partition lanes.
            attn_ps = psACC.tile([PB, HC], f32, name="attn_ps", tag="acc")
            NSB = SS // PB  # 8
            enc_dmas = []
            acc_a = vpool.tile([PB, H], f32, name="acc_a")
            for sb in range(NSB):
                et = encp.tile([PB, H], f32, name="et")
                enc_dmas.append(
                    nc.gpsimd.dma_start(et[:], enc_d[sb * PB:(sb + 1) * PB, :])
                )
                if sb == 0:
                    nc.vector.tensor_copy(acc_a[:], et[:])
                else:
                    nc.vector.tensor_add(acc_a[:], acc_a[:], et[:])
            for hb in range(HC):
                nc.tensor.matmul(
                    attn_ps[:, hb:hb + 1],
                    acc_a[:, hb * PB:(hb + 1) * PB],
                    scale_v[:],
                    start=(hb == 0),
                    stop=(hb == HC - 1),
                )
            attn_sb = vpool.tile([PB, HC], f32, name="attn_sb")
            nc.vector.tensor_copy(attn_sb[:], attn_ps[:])
            nc.sync.dma_start(
                attn_bo[:].rearrange("(p c) -> p c", p=PB), attn_sb[:]
            )
            nc.gpsimd.collective_compute(
                "AllReduce", ALU.add, replica_groups=RG,
                ins=[attn_bo[:].opt()], outs=[attn_rd[:].opt()],
            )
            # ---- During the attn AllReduce: load GRU weights, run gh
            # partials (independent of attn), and seed the logits PSUM with
            # the output bias.
            combt_s = wpool.tile([PB, 16 * PB], f32, name="combt_s")
            combt_dma = nc.gpsimd.dma_start(combt_s[:], combt_d[:])
            add_dep_helper(
                combt_dma.ins, enc_dmas[-1].ins,
                reason="stream order: enc before gru weights",
            )
            wih_s = wpool.tile([PB, 3 * H], f32, name="wih_s")
            wih_dma = nc.gpsimd.dma_start(wih_s[:], wiht_d[:])
            add_dep_helper(wih_dma.ins, combt_dma.ins, reason="stream order")
            whh_s = wpool.tile([PB, 3 * H], f32, name="whh_s")
            whh_dma = nc.gpsimd.dma_start(whh_s[:], whht_d[:])
            add_dep_helper(whh_dma.ins, wih_dma.ins, reason="stream order")
            h0c_v = vpool.tile([PB, 1], f32, name="h0c_v")
            nc.sync.dma_start(h0c_v[:], h0c_d[:].rearrange("(p o) -> p o", o=1))

            NRT_ = (3 * H) // PB  # 24 row-tiles per projection
            gh_ps = psT.tile([PB, NRT_], f32, name="gh_ps", tag="ghp")
            for r in range(NRT_):
                nc.tensor.matmul(
                    gh_ps[:, r:r + 1],
                    whh_s[:, r * PB:(r + 1) * PB],
                    h0c_v[:],
                    start=(r == 0), stop=(r == NRT_ - 1),
                )
            g_sb = vpool.tile([PB, 2 * NRT_], f32, name="g_sb")
            nc.vector.tensor_copy(g_sb[:, NRT_:], gh_ps[:])

            NSLC = (VC + 511) // 512  # 13 slices
            obc_r = spool.tile([1, VC], f32, name="obc_r", tag="bigrow1")
            nc.sync.dma_start(obc_r[:], obc_d[:].rearrange("(o f) -> o f", o=1))
            ps_rows = [
                psACC.tile([PB, 512], f32, name=f"ps_row{t}", tag=f"psrow{t}")
                for t in range(4)
            ]

            def slice_out(s, ncols):
                pb = 32 * (s % 4)
                return ps_rows[s // 4][pb:pb + 1, 0:ncols], (0, pb)

            # bias seeds each accumulation group (start=True)
            for s in range(NSLC):
                c0, c1 = s * 512, min((s + 1) * 512, VC)
                out_ap, tpos = slice_out(s, c1 - c0)
                nc.tensor.matmul(
                    out_ap, ones_1[:], obc_r[:, c0:c1],
                    start=True, stop=False, tile_position=tpos,
                )

            attn_v = vpool.tile([PB, HC], f32, name="attn_v")
            nc.sync.dma_start(attn_v[:], attn_rd[:].rearrange("(p c) -> p c", p=PB))

            # ---- Phase B: x chunk = relu(combT_chunk . cat(emb, attn) + b)
            emb_v = vpool.tile([PB, HC], f32, name="emb_v")
            nc.sync.dma_start(emb_v[:], emb_d[:].rearrange("(p c) -> p c", p=PB))
            combb_v = vpool.tile([PB, 1], f32, name="combb_v")
            nc.sync.dma_start(combb_v[:], combbc_d[:].rearrange("(p o) -> p o", o=1))

            x_ps = psT.tile([PB, 1], f32, name="x_ps", tag="tb")
            for k in range(16):
                rhs = emb_v[:, k:k + 1] if k < 8 else attn_v[:, k - 8:k - 7]
                nc.tensor.matmul(
                    x_ps[:], combt_s[:, k * PB:(k + 1) * PB], rhs,
                    start=(k == 0), stop=(k == 15),
                )
            x_v = vpool.tile([PB, 1], f32, name="x_v")
            nc.scalar.activation(x_v[:], x_ps[:], AF.Relu, bias=combb_v[:])

            # ---- Phase C: gi partials (need x from the attn collective)
            gi_ps = psACC.tile([PB, NRT_], f32, name="gi_ps", tag="acc")
            for r in range(NRT_):
                nc.tensor.matmul(
                    gi_ps[:, r:r + 1],
                    wih_s[:, r * PB:(r + 1) * PB],
                    x_v[:],
                    start=(r == 0), stop=(r == NRT_ - 1),
                )
            nc.vector.tensor_copy(g_sb[:, 0:NRT_], gi_ps[:])
            nc.sync.dma_start(
                g_bo[:].rearrange("(p c) -> p c", p=PB), g_sb[:]
            )
            nc.gpsimd.collective_compute(
                "AllReduce", ALU.add, replica_groups=RG,
                ins=[g_bo[:].opt()], outs=[g_rd[:].opt()],
            )

            # ---- Phase D: full gate math, replicated on every core
            g_all = vpool.tile([PB, 6, HC], f32, name="g_all")
            nc.sync.dma_start(
                g_all[:], g_rd[:].rearrange("(p s c) -> p s c", p=PB, s=6)
            )
            b_all = vpool.tile([PB, 4, HC], f32, name="b_all")
            nc.sync.dma_start(
                b_all[:], bvec_d[:].rearrange("(p s c) -> p s c", p=PB, s=4)
            )
            h0_v = vpool.tile([PB, HC], f32, name="h0_v")
            nc.sync.dma_start(h0_v[:], h0_d[:].rearrange("(p c) -> p c", p=PB))

            gi_r, gi_z, gi_n = g_all[:, 0, :], g_all[:, 1, :], g_all[:, 2, :]
            gh_r, gh_z, gh_n = g_all[:, 3, :], g_all[:, 4, :], g_all[:, 5, :]
            b_r, b_z = b_all[:, 0, :], b_all[:, 1, :]
            b_in, b_hn = b_all[:, 2, :], b_all[:, 3, :]

            t_r = spool.tile([PB, HC], f32, name="t_r")
            nc.vector.tensor_add(t_r[:], gi_r, gh_r)
            nc.vector.tensor_add(t_r[:], t_r[:], b_r)
            r_g = spool.tile([PB, HC], f32, name="r_g")
            nc.scalar.activation(r_g[:], t_r[:], AF.Sigmoid)

            t_z = spool.tile([PB, HC], f32, name="t_z")
            nc.vector.tensor_add(t_z[:], gi_z, gh_z)
            nc.vector.tensor_add(t_z[:], t_z[:], b_z)
            z_g = spool.tile([PB, HC], f32, name="z_g")
            nc.scalar.activation(z_g[:], t_z[:], AF.Sigmoid)

            t_n = spool.tile([PB, HC], f32, name="t_n")
            nc.vector.tensor_add(t_n[:], gh_n, b_hn)
            nc.vector.tensor_mul(t_n[:], r_g[:], t_n[:])
            nc.vector.tensor_add(t_n[:], t_n[:], gi_n)
            nc.vector.tensor_add(t_n[:], t_n[:], b_in)
            n_g = spool.tile([PB, HC], f32, name="n_g")
            nc.scalar.activation(n_g[:], t_n[:], AF.Tanh)

            d_t = spool.tile([PB, HC], f32, name="d_t")
            nc.vector.tensor_sub(d_t[:], h0_v[:], n_g[:])
            nc.vector.tensor_mul(d_t[:], z_g[:], d_t[:])
            hnew = spool.tile([PB, HC], f32, name="hnew")
            nc.vector.tensor_add(hnew[:], n_g[:], d_t[:])
            nc.sync.dma_start(outh_d[:].rearrange("(p c) -> p c", p=PB), hnew[:])

            # ---- Phase E: vocab-sharded output projection (the big matvec)
            # h chunk is the stationary operand (1 column), weight blocks
            # stream as the moving operand at N<=512.  The horizontal
            # [1, VC] result is spread over partitions {0,32,64,96} of four
            # [128, 512] PSUM tiles via tile_position col-tiling.
            # weight stream: 16 half-blocks (cols [0,LCOLS) and [LCOLS,VC))
            n_wt = 0
            for kb in range(HC):
                for (base, width, s_lo, s_hi) in (
                    (0, LCOLS, 0, LCOLS // 512),
                    (LCOLS, VC - LCOLS, LCOLS // 512, NSLC),
                ):
                    wt_t = wtp.tile([PB, width], f32, name="wt_t", tag="wt_t")
                    wdma = nc.gpsimd.dma_start(
                        wt_t[:],
                        wt_d[kb * PB:(kb + 1) * PB, base:base + width],
                    )
                    if n_wt < WT_BUFS:
                        add_dep_helper(
                            wdma.ins, whh_dma.ins,
                            reason="stream order: small weights before out_w",
                        )
                    n_wt += 1
                    for s in range(s_lo, s_hi):
                        c0, c1 = s * 512, min((s + 1) * 512, VC)
                        out_ap, tpos = slice_out(s, c1 - c0)
                        nc.tensor.matmul(
                            out_ap, hnew[:, kb:kb + 1],
                            wt_t[:, c0 - base:c1 - base],
                            start=False, stop=(kb == HC - 1),
                            tile_position=tpos,
                        )

            # per-slice maxes straight off PSUM
            smax = spool.tile([1, NSLC], f32, name="smax")
            for s in range(NSLC):
                c0, c1 = s * 512, min((s + 1) * 512, VC)
                out_ap, _ = slice_out(s, c1 - c0)
                nc.vector.tensor_reduce(
                    smax[:, s:s + 1], out_ap, axis=AX.X, op=ALU.max
                )
            mval = spool.tile([1, 1], f32, name="mval")
            nc.vector.tensor_reduce(mval[:], smax[:], axis=AX.X, op=ALU.max)
            negm = spool.tile([1, 1], f32, name="negm")
            nc.vector.tensor_scalar_mul(negm[:], mval[:], -1.0)

            # local sum(exp(logit - max)) per slice, accumulated off PSUM
            escr = spool.tile([1, 512], f32, name="escr")
            esl = spool.tile([1, NSLC], f32, name="esl")
            for s in range(NSLC):
                c0, c1 = s * 512, min((s + 1) * 512, VC)
                out_ap, _ = slice_out(s, c1 - c0)
                nc.scalar.activation(
                    escr[:, 0:c1 - c0], out_ap, AF.Exp, bias=negm[:],
                    accum_out=esl[:, s:s + 1],
                )
            sval = spool.tile([1, 1], f32, name="sval")
            nc.vector.tensor_reduce(sval[:], esl[:], axis=AX.X, op=ALU.add)

            nc.sync.dma_start(st_bo[0:1].rearrange("(o f) -> o f", o=1), mval[:])
            nc.sync.dma_start(st_bo[1:2].rearrange("(o f) -> o f", o=1), sval[:])
            nc.gpsimd.collective_compute(
                "AllGather", ALU.bypass, replica_groups=RG,
                ins=[st_bo[:].opt()], outs=[st_al[:].opt()],
            )

            # global logsumexp from the 8 (max, sumexp) pairs
            m_row = spool.tile([1, NCORES], f32, name="m_row")
            nc.sync.dma_start(
                m_row[:], st_al[:].rearrange("(r t) -> t r", t=2)[0:1, :]
            )
            s_row = spool.tile([1, NCORES], f32, name="s_row")
            nc.sync.dma_start(
                s_row[:], st_al[:].rearrange("(r t) -> t r", t=2)[1:2, :]
            )
            Mv = spool.tile([1, 1], f32, name="Mv")
            nc.vector.tensor_reduce(Mv[:], m_row[:], axis=AX.X, op=ALU.max)
            negM = spool.tile([1, 1], f32, name="negM")
            nc.vector.tensor_scalar_mul(negM[:], Mv[:], -1.0)
            wrow = spool.tile([1, NCORES], f32, name="wrow")
            nc.scalar.activation(wrow[:], m_row[:], AF.Exp, bias=negM[:])
            nc.vector.tensor_mul(wrow[:], wrow[:], s_row[:])
            Sv = spool.tile([1, 1], f32, name="Sv")
            nc.vector.tensor_reduce(Sv[:], wrow[:], axis=AX.X, op=ALU.add)
            lnS = spool.tile([1, 1], f32, name="lnS")
            nc.scalar.activation(lnS[:], Sv[:], AF.Ln)
            lse = spool.tile([1, 1], f32, name="lse")
            nc.vector.tensor_add(lse[:], lnS[:], Mv[:])
            nc.vector.tensor_scalar_mul(lse[:], lse[:], -1.0)

            final = spool.tile([1, VC], f32, name="final", tag="bigrow1")
            for s in range(NSLC):
                c0, c1 = s * 512, min((s + 1) * 512, VC)
                out_ap, _ = slice_out(s, c1 - c0)
                nc.scalar.activation(
                    final[:, c0:c1], out_ap, AF.Identity, bias=lse[:]
                )
            nc.sync.dma_start(outl_d[:].rearrange("(o f) -> o f", o=1), final[:])

    nc.compile()
    return nc


def _get_nc():
    if "nc" not in _CACHE:
        _CACHE["nc"] = build_nc()
    return _CACHE["nc"]


def _pmaj(v):
    """[H]-vector -> p-major layout: out[p*C + c] = v[c*PB + p]."""
    c = v.size // PB
    return np.ascontiguousarray(v.reshape(c, PB).T).reshape(-1)


def prep_in_maps(inputs):
    f = np.float32
    idx = int(np.asarray(inputs["input"]).reshape(-1)[0])
    emb = np.ascontiguousarray(np.asarray(inputs["embedding"])[idx], dtype=f)
    h0 = np.ascontiguousarray(np.asarray(inputs["hidden"]).reshape(H), dtype=f)
    enc = np.ascontiguousarray(np.asarray(inputs["encoder_outputs"]), dtype=f)
    combt = np.ascontiguousarray(np.asarray(inputs["comb_w"]).T, dtype=f)
    wiht = np.ascontiguousarray(np.asarray(inputs["w_ih"]).T, dtype=f)
    whht = np.ascontiguousarray(np.asarray(inputs["w_hh"]).T, dtype=f)
    b_ih = np.asarray(inputs["b_ih"], dtype=f)
    b_hh = np.asarray(inputs["b_hh"], dtype=f)
    # p-major-interleaved bias sections: bvec[p, s, c] = sec_s[c*PB + p]
    secs = [b_ih[:H] + b_hh[:H], b_ih[H:2 * H] + b_hh[H:2 * H],
            b_ih[2 * H:], b_hh[2 * H:]]
    bvec = np.ascontiguousarray(
        np.stack([sec.reshape(HC, PB) for sec in secs]).transpose(2, 0, 1)
    ).reshape(-1).astype(f)
    comb_b = np.asarray(inputs["comb_b"], dtype=f)
    wt_pad = np.zeros((H, VPAD), dtype=f)
    wt_pad[:, :V] = np.asarray(inputs["out_w"], dtype=f).T
    ob_pad = np.full((VPAD,), PAD_NEG, dtype=f)
    ob_pad[:V] = np.asarray(inputs["out_b"], dtype=f)

    in_maps = []
    for i in range(NCORES):
        # combt_pm[p, k*PB + m] = combT[k*PB + p, i*PB + m]
        ct = combt[:, i * PB:(i + 1) * PB].reshape(16, PB, PB)
        ct = np.ascontiguousarray(ct.transpose(1, 0, 2)).reshape(PB, 16 * PB)
        in_maps.append({
            "emb": _pmaj(emb),
            "h0": _pmaj(h0),
            "h0c": np.ascontiguousarray(h0[i * PB:(i + 1) * PB]),
            "enc": np.ascontiguousarray(enc[i * SS:(i + 1) * SS]),
            "combt": ct,
            "combbc": np.ascontiguousarray(comb_b[i * PB:(i + 1) * PB]),
            "wiht": np.ascontiguousarray(wiht[i * PB:(i + 1) * PB]),
            "whht": np.ascontiguousarray(whht[i * PB:(i + 1) * PB]),
            "bvec": bvec,
            "wt": np.ascontiguousarray(wt_pad[:, i * VC:(i + 1) * VC]),
            "obc": np.ascontiguousarray(ob_pad[i * VC:(i + 1) * VC]),
        })
    return in_maps


def assemble_outputs(results):
    logits = np.concatenate(
        [np.asarray(results[i]["out_l"]).reshape(-1) for i in range(NCORES)]
    )[:V]
    out = np.ascontiguousarray(logits, dtype=np.float32)[None, :]
    h_pm = np.asarray(results[0]["out_h"]).reshape(PB, HC)
    h_new = np.ascontiguousarray(h_pm.T, dtype=np.float32).reshape(-1)[None, None, :]
    attn_w = np.full((1, S), np.float32(1.0 / S), dtype=np.float32)
    return out, h_new, attn_w


def run_traced(inputs):
    """Run on HW with NTFF profiling; returns (outputs_tuple, exec_time_ns)."""
    from concourse.bass_utils import run_bass_kernel_spmd
    nc = _get_nc()
    in_maps = prep_in_maps(inputs)
    br = run_bass_kernel_spmd(
        nc, in_maps, core_ids=list(range(NCORES)), trace=True
    )
    return assemble_outputs(br.results), br.exec_time_ns


def kernel(**inputs):
    from concourse.bass_utils import run_bass_kernel_spmd
    nc = _get_nc()
    in_maps = prep_in_maps(inputs)
    br = run_bass_kernel_spmd(nc, in_maps, core_ids=list(range(NCORES)))
    return assemble_outputs(br.results)
